# revision 33
# baseline (speedup 1.0000x reference)
"""Single-head attention (B=8, S=2048, D=512) on 8 TRN2 NeuronCores.

Sharding: data-parallel over batch — core i computes batch element i
entirely locally (no collectives). Host-side prep is layout only
(transpose/reshape of the f32 shards); all compute runs on-device.

Math (per core, x = x[b] of shape [S, D]):
  Q^T[e,s] = sum_d WqT[d,e] xT[d,s] + bq[e]      (f32r matmul: x/W loaded as
                                                   f32r, no casts, fp22 MACs)
  K^T, V analogous; V kept as [s,e] bf16.
  S^T[k,q] = sum_e K^T[e,k] Q^T[e,q]             (bf16, transposed layout)
  E = exp(S^T / sqrt(D))                          (ScalarE -> bf16, no max-sub:
                                                   scores are O(5))
  denom[q] = sum_k E[k,q]: DVE partial sums + all-ones matmul partition fold
  ctx^T[e,q] = sum_k V[k,e] E[k,q]; normalized by 1/denom during psum->sbuf
  out[s,o] = sum_e ctx^T[e,s] WoT[e,o] + bo      -> DMA to DRAM

Scheduling notes (vs the 225 us baseline; this version ~196 us):
  - DMA descriptors of one dma_start already spread across all 16 queues;
    the limiter is the ~0.65-1 us per-dma_start issue cost on the in-order
    issuing engine (SP), plus a ~3.4 us cold DGE init on the first one.
    So: fewest possible dma_starts (one per 128-partition chunk, biases
    consolidated host-side), in priority order Wq, x[sb0], biases, Wk,
    x[sb1], Wv, x[sb2..3], Wo — the first projection starts at ~18 us
    instead of after the full 8 MB load.
  - f32r inputs: no cast step between DMA and the projection matmuls
    (also drops rel_err from 0.0077 to 0.0044 — fp22 MACs beat bf16).
  - PE warmup chain on memset ones bridges the DMA wait so the HAM never
    sees a >3.4 us idle gap (which would re-throttle the PE clock).
  - x tiles per-(dc,sb): tile-granular dependency tracking would otherwise
    serialize the first projection on the whole x load.
  - V interleaved per s-block; V/out biases ride DVE adds, not PE matmuls.
  - exp output tiles are per-kc so ctx matmuls don't over-serialize.
  - PSUM banks: psA=3/psS=3/psC=2. psA=2 cost ~4.5 us in WAR stalls that
    hide inside matmul slice durations (not PE gaps); psS=2 stalls the
    scores pipeline catastrophically.
  - fp8 was evaluated and rejected: DoubleRow measures 216 ns/instr on HW
    (2x bf16 per MAC, not the cost model's 4x), so a hi/lo-split fp8
    scheme is slower than bf16, and single fp8 fails the 2e-2 gate
    (3.4-5.4% measured in simulation).
"""

import sys

if "/opt/trn_rl_repo" not in sys.path:
    sys.path.insert(0, "/opt/trn_rl_repo")

import math

import numpy as np

import concourse.bass as bass
import concourse.mybir as mybir
import concourse.tile as tile

from concourse import bacc
from concourse.tile import TileContext

N_CORES = 8
S = 2048
D = 512
DO = 512

P = 128          # partition tile
F = 512          # free-dim tile (psum bank = 512 f32)
DC = D // P      # 4 contraction chunks over d
EC = D // P      # 4 chunks over e
SC = S // P      # 16 chunks over s (=k)
QB = S // F      # 4 q blocks of 512
KC = S // P      # 16 k chunks

F32 = mybir.dt.float32
F32R = mybir.dt.float32r
BF16 = mybir.dt.bfloat16

_SCALE = 1.0 / math.sqrt(D)


def build():
    nc = bacc.Bacc(None)

    # packed layouts: d-chunks side-by-side in the free dim so each tensor
    # (or x s-block) is ONE dma_start with 8 KB descriptors
    xT_e = nc.dram_tensor("xTp", [QB, P, DC * F], F32R, kind="ExternalInput")
    WqT_e = nc.dram_tensor("WqTp", [P, DC * D], F32R, kind="ExternalInput")
    WkT_e = nc.dram_tensor("WkTp", [P, DC * D], F32R, kind="ExternalInput")
    WvT_e = nc.dram_tensor("WvTp", [P, DC * D], F32R, kind="ExternalInput")
    WoT_e = nc.dram_tensor("WoTp", [P, DC * DO], F32, kind="ExternalInput")
    bq_e = nc.dram_tensor("bq_pc", [P, DC], F32, kind="ExternalInput")
    bk_e = nc.dram_tensor("bk_pc", [P, DC], F32, kind="ExternalInput")
    bv_e = nc.dram_tensor("bv_row", [1, D], F32, kind="ExternalInput")
    bo_e = nc.dram_tensor("bo_row", [1, DO], F32, kind="ExternalInput")
    out_e = nc.dram_tensor("out", [S, DO], F32, kind="ExternalOutput")

    with TileContext(nc) as tc:
        with (
            tc.tile_pool(name="io", bufs=2) as io,
            tc.tile_pool(name="persist", bufs=1) as ps,
            tc.tile_pool(name="expp", bufs=2) as expp,
            tc.tile_pool(name="recipp", bufs=2) as recipp,
            tc.tile_pool(name="outp", bufs=3) as outp,
            tc.tile_pool(name="psA", bufs=3, space="PSUM") as psA,
            tc.tile_pool(name="psS", bufs=3, space="PSUM") as psS,
            tc.tile_pool(name="psC", bufs=2, space="PSUM") as psC,
        ):
            # ---- constants + PE warmup (hides HAM clock ramp) -----------
            # gpsimd boots first -> earliest possible PE warmup start.
            # Warmup matmuls use a FULL 128-deep contraction: broadcast
            # (contraction=1) matmuls light up 1/128 of the array and the
            # HAM never ramps the clock off them.
            ones1 = ps.tile([1, P], BF16, tag="ones1", name="ones1")
            nc.gpsimd.memset(ones1[:], 1.0)
            ones128 = ps.tile([P, P], BF16, tag="ones128", name="ones128")
            nc.vector.memset(ones128[:], 1.0)
            warm = ps.tile([P, F], BF16, tag="warm", name="warm")
            nc.vector.memset(warm[:], 0.5)
            for i in range(34):
                pw = psA.tile([P, F], F32, tag="psA", name="psA")
                nc.tensor.matmul(pw[:], ones128[:], warm[:], start=True, stop=True)

            # ---- persistent operand tiles (f32r, packed, no casts) ------
            # One tile per tensor / per x s-block; d-chunks side-by-side in
            # the free dim (host packs them), sliced per-chunk at matmul time
            xA = [ps.tile([P, DC * F], F32R, tag=f"xA{sb}", name=f"xA{sb}") for sb in range(QB)]
            WqA = ps.tile([P, DC * D], F32R, tag="WqA", name="WqA")
            WkA = ps.tile([P, DC * D], F32R, tag="WkA", name="WkA")
            WvA = ps.tile([P, DC * D], F32R, tag="WvA", name="WvA")
            WoA = ps.tile([P, DC * DO], BF16, tag="WoA", name="WoA")
            xT = [[xA[sb][:, dc * F : (dc + 1) * F] for sb in range(QB)] for dc in range(DC)]
            WqT = [WqA[:, dc * D : (dc + 1) * D] for dc in range(DC)]
            WkT = [WkA[:, dc * D : (dc + 1) * D] for dc in range(DC)]
            WvT = [WvA[:, dc * D : (dc + 1) * D] for dc in range(DC)]
            WoT = [WoA[:, dc * DO : (dc + 1) * DO] for dc in range(DC)]

            # All loads on SP in priority order (per-queue FIFO): the first
            # projection needs only Wq + x[sb0] + bq.
            nc.sync.dma_start(WqA[:], WqT_e[:, :])
            nc.sync.dma_start(xA[0][:], xT_e[0])

            bqc = ps.tile([P, DC], F32, tag="bqc", name="bqc")
            bkc = ps.tile([P, DC], F32, tag="bkc", name="bkc")
            nc.sync.dma_start(bqc[:], bq_e[:, :])
            nc.sync.dma_start(bkc[:], bk_e[:, :])
            bvl = io.tile([1, D], F32, tag="brow", name="brow")
            nc.sync.dma_start(bvl[:], bv_e[:, :])
            bvr = ps.tile([1, D], BF16, tag="bvr", name="bvr")
            nc.vector.tensor_copy(bvr[:], bvl[:])
            bol = io.tile([1, DO], F32, tag="brow", name="brow")
            nc.sync.dma_start(bol[:], bo_e[:, :])
            bor = ps.tile([1, DO], BF16, tag="bor", name="bor")
            nc.vector.tensor_copy(bor[:], bol[:])

            nc.sync.dma_start(WkA[:], WkT_e[:, :])
            nc.sync.dma_start(xA[1][:], xT_e[1])
            nc.sync.dma_start(WvA[:], WvT_e[:, :])
            for sb in range(2, QB):
                nc.sync.dma_start(xA[sb][:], xT_e[sb])
            # Wo: f32 load + bf16 cast (out-proj lhsT ctxT is bf16; mixed
            # f32r x bf16 matmuls are not supported)
            wl = ps.tile([P, DC * DO], F32, tag="wol", name="wol")
            nc.sync.dma_start(wl[:], WoT_e[:, :])
            nc.vector.tensor_copy(WoA[:], wl[:])

            # broadcast bias rows to all 128 partitions; psum -> sbuf copies
            # ride DVE (ACT is busy issuing nothing, but keeping these off
            # psA's consumer chain matters: the pool reuse gates the first
            # projection matmul)
            bvb = ps.tile([P, D], F32, tag="bvb", name="bvb")
            bob = ps.tile([P, DO], F32, tag="bob", name="bob")
            pb = psA.tile([P, D], F32, tag="psA", name="psA")
            nc.tensor.matmul(pb[:], ones1[:], bvr[:], start=True, stop=True)
            nc.vector.tensor_copy(bvb[:], pb[:])
            pb = psA.tile([P, DO], F32, tag="psA", name="psA")
            nc.tensor.matmul(pb[:], ones1[:], bor[:], start=True, stop=True)
            nc.vector.tensor_copy(bob[:], pb[:])

            # ---- QKV projections (f32r) ---------------------------------
            QT = [ps.tile([P, S], BF16, tag=f"QT{ec}", name=f"QT{ec}") for ec in range(EC)]
            KT = [ps.tile([P, S], BF16, tag=f"KT{ec}", name=f"KT{ec}") for ec in range(EC)]
            V = [ps.tile([P, D], BF16, tag=f"V{sc}", name=f"V{sc}") for sc in range(SC)]
            ctxT = [ps.tile([P, S], BF16, tag=f"ctxT{ec}", name=f"ctxT{ec}") for ec in range(EC)]

            for sb in range(QB):
                ss = slice(sb * F, (sb + 1) * F)
                for ec in range(EC):
                    es = slice(ec * P, (ec + 1) * P)
                    pq = psA.tile([P, F], F32, tag="psA", name="psA")
                    for dc in range(DC):
                        nc.tensor.matmul(
                            pq[:], WqT[dc][:, es], xT[dc][sb],
                            start=(dc == 0), stop=(dc == DC - 1),
                        )
                    nc.scalar.add(QT[ec][:, ss], pq[:], bqc[:, ec : ec + 1])
                    pk = psA.tile([P, F], F32, tag="psA", name="psA")
                    for dc in range(DC):
                        nc.tensor.matmul(
                            pk[:], WkT[dc][:, es], xT[dc][sb],
                            start=(dc == 0), stop=(dc == DC - 1),
                        )
                    nc.scalar.add(KT[ec][:, ss], pk[:], bkc[:, ec : ec + 1])
                for sj in range(DC):
                    sc = sb * DC + sj
                    js = slice(sj * P, (sj + 1) * P)
                    pv = psA.tile([P, D], F32, tag="psA", name="psA")
                    for dc in range(DC):
                        nc.tensor.matmul(
                            pv[:], xT[dc][sb][:, js], WvT[dc],
                            start=(dc == 0), stop=(dc == DC - 1),
                        )
                    nc.vector.tensor_add(V[sc][:], pv[:], bvb[:])

            # ---- attention, blocked over q ------------------------------
            for qb in range(QB):
                qs = slice(qb * F, (qb + 1) * F)
                eblk = [
                    expp.tile([P, F], BF16, tag=f"e{kc}", name=f"e{kc}")
                    for kc in range(KC)
                ]
                denp = recipp.tile([P, F], F32, tag="denp", name="denp")
                for kc in range(KC):
                    ks = slice(kc * P, (kc + 1) * P)
                    pss = psS.tile([P, F], F32, tag="psS", name="psS")
                    for ec in range(EC):
                        nc.tensor.matmul(
                            pss[:], KT[ec][:, ks], QT[ec][:, qs],
                            start=(ec == 0), stop=(ec == EC - 1),
                        )
                    nc.scalar.activation(
                        eblk[kc][:], pss[:],
                        mybir.ActivationFunctionType.Exp, scale=_SCALE,
                    )
                    # partial softmax denominator on DVE (keeps PE free)
                    if kc == 0:
                        nc.vector.tensor_copy(denp[:], eblk[kc][:])
                    else:
                        nc.vector.tensor_add(denp[:], denp[:], eblk[kc][:])

                denb = recipp.tile([P, F], BF16, tag="denb", name="denb")
                nc.vector.tensor_copy(denb[:], denp[:])
                pd = psS.tile([P, F], F32, tag="psS", name="psS")
                nc.tensor.matmul(pd[:], ones128[:], denb[:], start=True, stop=True)
                recip = recipp.tile([P, F], F32, tag="recip", name="recip")
                nc.vector.reciprocal_approx_fast(recip[:], pd[:])

                for ec in range(EC):
                    es = slice(ec * P, (ec + 1) * P)
                    pc = psC.tile([P, F], F32, tag="psC", name="psC")
                    for kc in range(KC):
                        nc.tensor.matmul(
                            pc[:], V[kc][:, es], eblk[kc][:],
                            start=(kc == 0), stop=(kc == KC - 1),
                        )
                    nc.vector.tensor_mul(ctxT[ec][:, qs], pc[:], recip[:])

                for sj in range(QB):
                    s0 = qb * F + sj * P
                    po = psA.tile([P, DO], F32, tag="psA", name="psA")
                    for ec in range(EC):
                        nc.tensor.matmul(
                            po[:], ctxT[ec][:, s0 : s0 + P], WoT[ec],
                            start=(ec == 0), stop=(ec == EC - 1),
                        )
                    ot = outp.tile([P, DO], F32, tag="out", name="outtile")
                    nc.vector.tensor_add(ot[:], po[:], bob[:])
                    nc.sync.dma_start(out_e[s0 : s0 + P, :], ot[:])

    nc.compile()
    return nc


_NC = None


def _get_nc():
    global _NC
    if _NC is None:
        _NC = build()
    return _NC


def _make_in_maps(x, Wq, bq, Wk, bk, Wv, bv, Wo, bo):
    # Layout-only host prep: per-core shard = one batch element, transposed
    # weight/activation layouts (f32 throughout; f32r reads the same bits).
    def packw(W):
        # [e, d] -> W.T [d, e] -> [p, dc*D + e]: d-chunks side-by-side
        return np.ascontiguousarray(
            np.asarray(W, np.float32).T.reshape(DC, P, -1).transpose(1, 0, 2).reshape(P, -1)
        )

    WqT = packw(Wq)
    WkT = packw(Wk)
    WvT = packw(Wv)
    WoT = packw(Wo)
    bq_pc = np.ascontiguousarray(np.asarray(bq, np.float32).reshape(DC, P).T)
    bk_pc = np.ascontiguousarray(np.asarray(bk, np.float32).reshape(DC, P).T)
    bv_row = np.ascontiguousarray(np.asarray(bv, np.float32).reshape(1, D))
    bo_row = np.ascontiguousarray(np.asarray(bo, np.float32).reshape(1, DO))
    in_maps = []
    for i in range(N_CORES):
        in_maps.append(
            {
                "xTp": np.ascontiguousarray(
                    np.asarray(x[i], np.float32).T.reshape(DC, P, QB, F)
                    .transpose(2, 1, 0, 3).reshape(QB, P, DC * F)
                ),
                "WqTp": WqT,
                "WkTp": WkT,
                "WvTp": WvT,
                "WoTp": WoT,
                "bq_pc": bq_pc,
                "bk_pc": bk_pc,
                "bv_row": bv_row,
                "bo_row": bo_row,
            }
        )
    return in_maps


def run(inputs, trace=False):
    """Compile (cached) + run on cores 0-7. Returns (output, BassKernelResults)."""
    from concourse.bass_utils import run_bass_kernel_spmd

    nc = _get_nc()
    in_maps = _make_in_maps(**inputs)
    res = run_bass_kernel_spmd(
        nc, in_maps, core_ids=list(range(N_CORES)), trace=trace
    )
    out = np.stack([res.results[i]["out"] for i in range(N_CORES)], axis=0)
    return out.astype(np.float32), res


def kernel(**inputs) -> np.ndarray:
    out, _ = run(inputs, trace=False)
    return out


# revision 34
# speedup vs baseline: 1.0029x; 1.0029x over previous
"""Single-head attention (B=8, S=2048, D=512) on 8 TRN2 NeuronCores.

Sharding: data-parallel over batch — core i computes batch element i
entirely locally (no collectives). Host-side prep is layout only
(transpose/reshape of the f32 shards); all compute runs on-device.

Math (per core, x = x[b] of shape [S, D]):
  Q^T[e,s] = sum_d WqT[d,e] xT[d,s] + bq[e]      (f32r matmul: x/W loaded as
                                                   f32r, no casts, fp22 MACs)
  K^T, V analogous; V kept as [s,e] bf16.
  S^T[k,q] = sum_e K^T[e,k] Q^T[e,q]             (bf16, transposed layout)
  E = exp(S^T / sqrt(D))                          (ScalarE -> bf16, no max-sub:
                                                   scores are O(5))
  denom[q] = sum_k E[k,q]: DVE partial sums + all-ones matmul partition fold
  ctx^T[e,q] = sum_k V[k,e] E[k,q]; normalized by 1/denom during psum->sbuf
  out[s,o] = sum_e ctx^T[e,s] WoT[e,o] + bo      -> DMA to DRAM

Scheduling notes (vs the 225 us baseline; this version ~196 us):
  - DMA descriptors of one dma_start already spread across all 16 queues;
    the limiter is the ~0.65-1 us per-dma_start issue cost on the in-order
    issuing engine (SP), plus a ~3.4 us cold DGE init on the first one.
    So: fewest possible dma_starts (one per 128-partition chunk, biases
    consolidated host-side), in priority order Wq, x[sb0], biases, Wk,
    x[sb1], Wv, x[sb2..3], Wo — the first projection starts at ~18 us
    instead of after the full 8 MB load.
  - f32r inputs: no cast step between DMA and the projection matmuls
    (also drops rel_err from 0.0077 to 0.0044 — fp22 MACs beat bf16).
  - PE warmup chain on memset ones bridges the DMA wait so the HAM never
    sees a >3.4 us idle gap (which would re-throttle the PE clock).
  - x tiles per-(dc,sb): tile-granular dependency tracking would otherwise
    serialize the first projection on the whole x load.
  - V interleaved per s-block; V/out biases ride DVE adds, not PE matmuls.
  - exp output tiles are per-kc so ctx matmuls don't over-serialize.
  - PSUM banks: psA=3/psS=3/psC=2. psA=2 cost ~4.5 us in WAR stalls that
    hide inside matmul slice durations (not PE gaps); psS=2 stalls the
    scores pipeline catastrophically.
  - fp8 was evaluated and rejected: DoubleRow measures 216 ns/instr on HW
    (2x bf16 per MAC, not the cost model's 4x), so a hi/lo-split fp8
    scheme is slower than bf16, and single fp8 fails the 2e-2 gate
    (3.4-5.4% measured in simulation).
"""

import sys

if "/opt/trn_rl_repo" not in sys.path:
    sys.path.insert(0, "/opt/trn_rl_repo")

import math

import numpy as np

import concourse.bass as bass
import concourse.mybir as mybir
import concourse.tile as tile

from concourse import bacc
from concourse.tile import TileContext

N_CORES = 8
S = 2048
D = 512
DO = 512

P = 128          # partition tile
F = 512          # free-dim tile (psum bank = 512 f32)
DC = D // P      # 4 contraction chunks over d
EC = D // P      # 4 chunks over e
SC = S // P      # 16 chunks over s (=k)
QB = S // F      # 4 q blocks of 512
KC = S // P      # 16 k chunks

F32 = mybir.dt.float32
F32R = mybir.dt.float32r
BF16 = mybir.dt.bfloat16

_SCALE = 1.0 / math.sqrt(D)


def build():
    nc = bacc.Bacc(None)

    # packed layouts: d-chunks side-by-side in the free dim so each tensor
    # (or x s-block) is ONE dma_start with 8 KB descriptors
    xT_e = nc.dram_tensor("xTp", [QB, P, DC * F], F32R, kind="ExternalInput")
    WqT_e = nc.dram_tensor("WqTp", [P, DC * D], F32R, kind="ExternalInput")
    WkT_e = nc.dram_tensor("WkTp", [P, DC * D], F32R, kind="ExternalInput")
    WvT_e = nc.dram_tensor("WvTp", [P, DC * D], F32R, kind="ExternalInput")
    WoT_e = nc.dram_tensor("WoTp", [P, DC * DO], F32, kind="ExternalInput")
    bq_e = nc.dram_tensor("bq_pc", [P, DC], F32, kind="ExternalInput")
    bk_e = nc.dram_tensor("bk_pc", [P, DC], F32, kind="ExternalInput")
    bv_e = nc.dram_tensor("bv_row", [1, D], F32, kind="ExternalInput")
    bo_e = nc.dram_tensor("bo_row", [1, DO], F32, kind="ExternalInput")
    out_e = nc.dram_tensor("out", [S, DO], F32, kind="ExternalOutput")

    with TileContext(nc) as tc:
        with (
            tc.tile_pool(name="io", bufs=2) as io,
            tc.tile_pool(name="persist", bufs=1) as ps,
            tc.tile_pool(name="expp", bufs=2) as expp,
            tc.tile_pool(name="recipp", bufs=2) as recipp,
            tc.tile_pool(name="outp", bufs=3) as outp,
            tc.tile_pool(name="psA", bufs=3, space="PSUM") as psA,
            tc.tile_pool(name="psS", bufs=3, space="PSUM") as psS,
            tc.tile_pool(name="psC", bufs=2, space="PSUM") as psC,
        ):
            # ---- constants + PE warmup (hides HAM clock ramp) -----------
            # gpsimd boots first -> earliest possible PE warmup start.
            # Warmup matmuls use a FULL 128-deep contraction: broadcast
            # (contraction=1) matmuls light up 1/128 of the array and the
            # HAM never ramps the clock off them.
            ones1 = ps.tile([1, P], BF16, tag="ones1", name="ones1")
            nc.gpsimd.memset(ones1[:], 1.0)
            ones128 = ps.tile([P, P], BF16, tag="ones128", name="ones128")
            nc.vector.memset(ones128[:], 1.0)
            warm = ps.tile([P, F], BF16, tag="warm", name="warm")
            nc.vector.memset(warm[:], 0.5)
            for i in range(40):
                pw = psA.tile([P, F], F32, tag="psA", name="psA")
                nc.tensor.matmul(pw[:], ones128[:], warm[:], start=True, stop=True)

            # ---- persistent operand tiles (f32r, packed, no casts) ------
            # One tile per tensor / per x s-block; d-chunks side-by-side in
            # the free dim (host packs them), sliced per-chunk at matmul time
            xA = [ps.tile([P, DC * F], F32R, tag=f"xA{sb}", name=f"xA{sb}") for sb in range(QB)]
            WqA = ps.tile([P, DC * D], F32R, tag="WqA", name="WqA")
            WkA = ps.tile([P, DC * D], F32R, tag="WkA", name="WkA")
            WvA = ps.tile([P, DC * D], F32R, tag="WvA", name="WvA")
            WoA = ps.tile([P, DC * DO], BF16, tag="WoA", name="WoA")
            xT = [[xA[sb][:, dc * F : (dc + 1) * F] for sb in range(QB)] for dc in range(DC)]
            WqT = [WqA[:, dc * D : (dc + 1) * D] for dc in range(DC)]
            WkT = [WkA[:, dc * D : (dc + 1) * D] for dc in range(DC)]
            WvT = [WvA[:, dc * D : (dc + 1) * D] for dc in range(DC)]
            WoT = [WoA[:, dc * DO : (dc + 1) * DO] for dc in range(DC)]

            # All loads on SP in priority order (per-queue FIFO): the first
            # projection needs only Wq + x[sb0] + bq.
            nc.sync.dma_start(WqA[:], WqT_e[:, :])
            nc.sync.dma_start(xA[0][:], xT_e[0])

            bqc = ps.tile([P, DC], F32, tag="bqc", name="bqc")
            bkc = ps.tile([P, DC], F32, tag="bkc", name="bkc")
            nc.sync.dma_start(bqc[:], bq_e[:, :])
            nc.sync.dma_start(bkc[:], bk_e[:, :])
            bvl = io.tile([1, D], F32, tag="brow", name="brow")
            nc.sync.dma_start(bvl[:], bv_e[:, :])
            bvr = ps.tile([1, D], BF16, tag="bvr", name="bvr")
            nc.vector.tensor_copy(bvr[:], bvl[:])
            bol = io.tile([1, DO], F32, tag="brow", name="brow")
            nc.sync.dma_start(bol[:], bo_e[:, :])
            bor = ps.tile([1, DO], BF16, tag="bor", name="bor")
            nc.vector.tensor_copy(bor[:], bol[:])

            nc.sync.dma_start(WkA[:], WkT_e[:, :])
            nc.sync.dma_start(xA[1][:], xT_e[1])
            nc.sync.dma_start(WvA[:], WvT_e[:, :])
            for sb in range(2, QB):
                nc.sync.dma_start(xA[sb][:], xT_e[sb])
            # Wo: f32 load + bf16 cast (out-proj lhsT ctxT is bf16; mixed
            # f32r x bf16 matmuls are not supported)
            wl = ps.tile([P, DC * DO], F32, tag="wol", name="wol")
            nc.sync.dma_start(wl[:], WoT_e[:, :])
            nc.vector.tensor_copy(WoA[:], wl[:])

            # broadcast bias rows to all 128 partitions; psum -> sbuf copies
            # ride DVE (ACT is busy issuing nothing, but keeping these off
            # psA's consumer chain matters: the pool reuse gates the first
            # projection matmul)
            bvb = ps.tile([P, D], F32, tag="bvb", name="bvb")
            bob = ps.tile([P, DO], F32, tag="bob", name="bob")
            pb = psA.tile([P, D], F32, tag="psA", name="psA")
            nc.tensor.matmul(pb[:], ones1[:], bvr[:], start=True, stop=True)
            nc.vector.tensor_copy(bvb[:], pb[:])
            pb = psA.tile([P, DO], F32, tag="psA", name="psA")
            nc.tensor.matmul(pb[:], ones1[:], bor[:], start=True, stop=True)
            nc.vector.tensor_copy(bob[:], pb[:])

            # ---- QKV projections (f32r) ---------------------------------
            QT = [ps.tile([P, S], BF16, tag=f"QT{ec}", name=f"QT{ec}") for ec in range(EC)]
            KT = [ps.tile([P, S], BF16, tag=f"KT{ec}", name=f"KT{ec}") for ec in range(EC)]
            V = [ps.tile([P, D], BF16, tag=f"V{sc}", name=f"V{sc}") for sc in range(SC)]
            ctxT = [ps.tile([P, S], BF16, tag=f"ctxT{ec}", name=f"ctxT{ec}") for ec in range(EC)]

            for sb in range(QB):
                ss = slice(sb * F, (sb + 1) * F)
                for ec in range(EC):
                    es = slice(ec * P, (ec + 1) * P)
                    pq = psA.tile([P, F], F32, tag="psA", name="psA")
                    for dc in range(DC):
                        nc.tensor.matmul(
                            pq[:], WqT[dc][:, es], xT[dc][sb],
                            start=(dc == 0), stop=(dc == DC - 1),
                        )
                    nc.scalar.add(QT[ec][:, ss], pq[:], bqc[:, ec : ec + 1])
                    pk = psA.tile([P, F], F32, tag="psA", name="psA")
                    for dc in range(DC):
                        nc.tensor.matmul(
                            pk[:], WkT[dc][:, es], xT[dc][sb],
                            start=(dc == 0), stop=(dc == DC - 1),
                        )
                    nc.scalar.add(KT[ec][:, ss], pk[:], bkc[:, ec : ec + 1])
                for sj in range(DC):
                    sc = sb * DC + sj
                    js = slice(sj * P, (sj + 1) * P)
                    pv = psA.tile([P, D], F32, tag="psA", name="psA")
                    for dc in range(DC):
                        nc.tensor.matmul(
                            pv[:], xT[dc][sb][:, js], WvT[dc],
                            start=(dc == 0), stop=(dc == DC - 1),
                        )
                    nc.vector.tensor_add(V[sc][:], pv[:], bvb[:])

            # ---- attention, blocked over q ------------------------------
            for qb in range(QB):
                qs = slice(qb * F, (qb + 1) * F)
                eblk = [
                    expp.tile([P, F], BF16, tag=f"e{kc}", name=f"e{kc}")
                    for kc in range(KC)
                ]
                denp = recipp.tile([P, F], F32, tag="denp", name="denp")
                for kc in range(KC):
                    ks = slice(kc * P, (kc + 1) * P)
                    pss = psS.tile([P, F], F32, tag="psS", name="psS")
                    for ec in range(EC):
                        nc.tensor.matmul(
                            pss[:], KT[ec][:, ks], QT[ec][:, qs],
                            start=(ec == 0), stop=(ec == EC - 1),
                        )
                    nc.scalar.activation(
                        eblk[kc][:], pss[:],
                        mybir.ActivationFunctionType.Exp, scale=_SCALE,
                    )
                    # partial softmax denominator on DVE (keeps PE free)
                    if kc == 0:
                        nc.vector.tensor_copy(denp[:], eblk[kc][:])
                    else:
                        nc.vector.tensor_add(denp[:], denp[:], eblk[kc][:])

                denb = recipp.tile([P, F], BF16, tag="denb", name="denb")
                nc.vector.tensor_copy(denb[:], denp[:])
                pd = psS.tile([P, F], F32, tag="psS", name="psS")
                nc.tensor.matmul(pd[:], ones128[:], denb[:], start=True, stop=True)
                recip = recipp.tile([P, F], F32, tag="recip", name="recip")
                nc.vector.reciprocal_approx_fast(recip[:], pd[:])

                for ec in range(EC):
                    es = slice(ec * P, (ec + 1) * P)
                    pc = psC.tile([P, F], F32, tag="psC", name="psC")
                    for kc in range(KC):
                        nc.tensor.matmul(
                            pc[:], V[kc][:, es], eblk[kc][:],
                            start=(kc == 0), stop=(kc == KC - 1),
                        )
                    nc.vector.tensor_mul(ctxT[ec][:, qs], pc[:], recip[:])

                for sj in range(QB):
                    s0 = qb * F + sj * P
                    po = psA.tile([P, DO], F32, tag="psA", name="psA")
                    for ec in range(EC):
                        nc.tensor.matmul(
                            po[:], ctxT[ec][:, s0 : s0 + P], WoT[ec],
                            start=(ec == 0), stop=(ec == EC - 1),
                        )
                    ot = outp.tile([P, DO], F32, tag="out", name="outtile")
                    nc.vector.tensor_add(ot[:], po[:], bob[:])
                    nc.sync.dma_start(out_e[s0 : s0 + P, :], ot[:])

    nc.compile()
    return nc


_NC = None


def _get_nc():
    global _NC
    if _NC is None:
        _NC = build()
    return _NC


def _make_in_maps(x, Wq, bq, Wk, bk, Wv, bv, Wo, bo):
    # Layout-only host prep: per-core shard = one batch element, transposed
    # weight/activation layouts (f32 throughout; f32r reads the same bits).
    def packw(W):
        # [e, d] -> W.T [d, e] -> [p, dc*D + e]: d-chunks side-by-side
        return np.ascontiguousarray(
            np.asarray(W, np.float32).T.reshape(DC, P, -1).transpose(1, 0, 2).reshape(P, -1)
        )

    WqT = packw(Wq)
    WkT = packw(Wk)
    WvT = packw(Wv)
    WoT = packw(Wo)
    bq_pc = np.ascontiguousarray(np.asarray(bq, np.float32).reshape(DC, P).T)
    bk_pc = np.ascontiguousarray(np.asarray(bk, np.float32).reshape(DC, P).T)
    bv_row = np.ascontiguousarray(np.asarray(bv, np.float32).reshape(1, D))
    bo_row = np.ascontiguousarray(np.asarray(bo, np.float32).reshape(1, DO))
    in_maps = []
    for i in range(N_CORES):
        in_maps.append(
            {
                "xTp": np.ascontiguousarray(
                    np.asarray(x[i], np.float32).T.reshape(DC, P, QB, F)
                    .transpose(2, 1, 0, 3).reshape(QB, P, DC * F)
                ),
                "WqTp": WqT,
                "WkTp": WkT,
                "WvTp": WvT,
                "WoTp": WoT,
                "bq_pc": bq_pc,
                "bk_pc": bk_pc,
                "bv_row": bv_row,
                "bo_row": bo_row,
            }
        )
    return in_maps


def run(inputs, trace=False):
    """Compile (cached) + run on cores 0-7. Returns (output, BassKernelResults)."""
    from concourse.bass_utils import run_bass_kernel_spmd

    nc = _get_nc()
    in_maps = _make_in_maps(**inputs)
    res = run_bass_kernel_spmd(
        nc, in_maps, core_ids=list(range(N_CORES)), trace=trace
    )
    out = np.stack([res.results[i]["out"] for i in range(N_CORES)], axis=0)
    return out.astype(np.float32), res


def kernel(**inputs) -> np.ndarray:
    out, _ = run(inputs, trace=False)
    return out
